# revision 23
# baseline (speedup 1.0000x reference)
"""Trainium2 Bass kernel for a dense pre-norm transformer block.

Reference computation (per batch element, fp32):
    nx = LN(x; g1, beta1);  per-head q/k/v proj (shared [64,64] weights);
    causal softmax(QK^T / sqrt(1024));  out proj Wo + residual;
    nx2 = LN(x; g2, beta2);  x + relu(nx2 @ W1 + b1) @ W2 + b2.

Distribution: pure data parallel - batch B=8, one batch element per
NeuronCore, weights replicated, no collectives.

Per-core kernel strategy (matmul operands in bf16, fp32 PSUM accum):
  - LN affine (g, beta) folded into projection weights on the host.
  - Q^T/K^T per head-pair with block-diagonal [128,128] weights.
  - Scores transposed (S^T[k,q]); softmax denominator l from a ones
    column appended to the PV stationary; no max pass (scores/32 are
    O(0.1)).  exp on ACT; causal mask by 0/1 multiply on diagonal
    chunks (DVE, bf16 2x); 1/l = exp(-ln l) on ACT so the whole
    attention phase stays inside the natural_log_exp table set.
  - V never materialized: U_h = [nx_h | 1]^T @ P_h^T gives values and
    denominator in one PSUM accumulation; Wv@Wo fused on host to wvo.
  - PSUM budget (8 banks): scores+qk share one [128,2,1024] tile
    (4 banks) + two [65,1024] U accumulators (4 banks).
  - attn-out accumulated over pairs per 128-token group, immediately
    followed by residual + LN2 + transposes for that group so FFN W1
    starts while attn-out of later groups still runs.
  - FFN: h1 kept f-major in bf16; W2 resident in SBUF; W2 loop runs
    token-group-major so outputs drain early (no serial tail).
"""

import functools
import math
import os
from contextlib import ExitStack

import ml_dtypes
import numpy as np

import concourse.bass as bass
import concourse.tile as tile
from concourse import bacc, mybir
from concourse.bass_utils import run_bass_kernel_spmd

F32 = mybir.dt.float32
BF16 = mybir.dt.bfloat16
AF = mybir.ActivationFunctionType
AL = mybir.AluOpType

B, S, E, H, D, F = 8, 1024, 1024, 16, 64, 4096
P = 128
NT = S // P            # 8 token tiles
NPAIR = H // 2         # 8 head pairs
NF = F // P            # 32 f tiles
NE = E // P            # 8 e tiles
EPS = 1e-5
SCALE = 1.0 / math.sqrt(float(E))  # reference scales scores by sqrt(embed)


def _build_program():
    nc = bacc.Bacc("TRN2")

    xd = nc.dram_tensor("x", (S, E), F32, kind="ExternalInput")
    wqd = nc.dram_tensor("wqblk", (NPAIR, P, P), BF16, kind="ExternalInput")
    wkd = nc.dram_tensor("wkblk", (NPAIR, P, P), BF16, kind="ExternalInput")
    wvod = nc.dram_tensor("wvo", (NPAIR, P, E), BF16, kind="ExternalInput")
    w1d = nc.dram_tensor("w1", (NF, P, NE * P), BF16, kind="ExternalInput")
    w2d = nc.dram_tensor("w2", (P, NF, E), BF16, kind="ExternalInput")
    maskd = nc.dram_tensor("masks", (P, P), BF16, kind="ExternalInput")
    identd = nc.dram_tensor("ident", (P, P), BF16, kind="ExternalInput")
    outd = nc.dram_tensor("out", (S, E), F32, kind="ExternalOutput")

    reps = int(os.environ.get("KREP", "1"))
    with tile.TileContext(nc) as tc:
        for _ in range(reps):
            _emit(nc, tc, xd, wqd, wkd, wvod, w1d, w2d, maskd, identd, outd)
    nc.compile()
    return nc


def _emit(nc, tc, xd, wqd, wkd, wvod, w1d, w2d, maskd, identd, outd):
    xv = xd.rearrange("(t p) e -> p t e", p=P)
    ov = outd.rearrange("(t p) e -> p t e", p=P)

    with tc.tile_pool(name="consts", bufs=1) as consts, \
            tc.tile_pool(name="persist", bufs=1) as persist, \
            tc.tile_pool(name="work", bufs=1) as work:
        x_all = persist.tile([P, NT, E], F32)
        nc.sync.dma_start(out=x_all[:, 0, :], in_=xv[:, 0, :])
        ident = consts.tile([P, P], BF16)
        nc.sync.dma_start(out=ident, in_=identd[:, :])
        for t in range(1, NT):
            nc.sync.dma_start(out=x_all[:, t, :], in_=xv[:, t, :])
        masks = consts.tile([P, P], BF16)
        nc.sync.dma_start(out=masks, in_=maskd[:, :])
        wqsb = consts.tile([P, NPAIR, P], BF16)
        nc.sync.dma_start(out=wqsb, in_=wqd.rearrange("b k m -> k b m"))
        wksb = consts.tile([P, NPAIR, P], BF16)
        nc.sync.dma_start(out=wksb, in_=wkd.rearrange("b k m -> k b m"))
        epssb = consts.tile([P, 1], F32)
        nc.vector.memset(epssb, EPS)

        nx2T = persist.tile([P, NE, S], BF16)

        with tc.tile_pool(name="attn_sb", bufs=1) as asb:
            u_all = asb.tile([P, NPAIR, S], BF16)
            aug = asb.tile([P, NT, H * (D + 1)], BF16)
            nxT = asb.tile([P, NE, S], BF16)
            # attn-out weights prefetch on the Pool queue (idle early)
            wvosb = asb.tile([P, NPAIR, E], BF16)
            nc.gpsimd.dma_start(out=wvosb,
                                in_=wvod.rearrange("b k e -> k b e"))

            # ---------- stage 1: LN1 + transposes ------------------------
            # LN writes only the head-interleaved aug (ACT); per-head
            # [128,64] transposes write head parity at PSUM partition
            # offset 0/64, giving pair-stacked nxT blocks directly.
            with tc.tile_pool(name="psum_t1", bufs=1, space="PSUM") as pt1:
                for t in range(NT):
                    _layernorm(nc, work, x_all[:, t, :], None, epssb,
                               aug_ap=aug[:, t, :])
                    for half in range(2):
                        tp = pt1.tile([P, 512], BF16, tag="tp1", bufs=2)
                        for i in range(4):
                            b = 4 * half + i
                            for par in range(2):
                                h = 2 * b + par
                                nc.tensor.transpose(
                                    tp[par * D:par * D + D,
                                       i * P:(i + 1) * P],
                                    aug[:, t,
                                        (D + 1) * h:(D + 1) * h + D],
                                    ident)
                        nc.vector.tensor_copy(
                            out=nxT[:, 4 * half:4 * half + 4,
                                    t * P:(t + 1) * P],
                            in_=tp.rearrange("p (b m) -> p b m", b=4))

            # ---------- stage 2: attention per head pair ------------------
            with tc.tile_pool(name="psum_at", bufs=1, space="PSUM") as pat:
                qk_sb = {}

                def emit_qkproj(pp, sp):
                    # q/k projections into the score psum tile; k copy on
                    # ACT (fast PSUM path), q copy on DVE, so the next
                    # pair's first score matmul unblocks quickly.
                    qsb = asb.tile([P, S], BF16, tag="qsb", bufs=2,
                                   name=f"qsb{pp % 2}")
                    ksb = asb.tile([P, S], BF16, tag="ksb", bufs=2,
                                   name=f"ksb{pp % 2}")
                    for qk, wsb in ((0, wqsb), (1, wksb)):
                        for c in range(2):
                            nc.tensor.matmul(
                                sp[:, qk, c * 512:(c + 1) * 512],
                                wsb[:, pp, :],
                                nxT[:, pp, c * 512:(c + 1) * 512],
                                start=True, stop=True)
                    nc.scalar.activation(out=ksb, in_=sp[:, 1, :],
                                         func=AF.Copy)
                    nc.vector.tensor_copy(out=qsb, in_=sp[:, 0, :])
                    qk_sb[pp] = (qsb, ksb)

                sp = pat.tile([P, 2, S], F32, tag="sp", bufs=1)
                emit_qkproj(0, sp)
                for p in range(NPAIR):
                    qsb, ksb = qk_sb.pop(p)
                    ups = [pat.tile([D + 1, S], F32, tag=f"ups{i}", bufs=1,
                                    name=f"ups{i}")
                           for i in range(2)]
                    for t in range(NT):
                        lo = t * P
                        for par in range(2):
                            ks = ksb[par * D:par * D + D, t * P:(t + 1) * P]
                            if lo < 512:
                                nc.tensor.matmul(
                                    sp[:, par, lo:512], ks,
                                    qsb[par * D:par * D + D, lo:512],
                                    start=True, stop=True)
                                nc.tensor.matmul(
                                    sp[:, par, 512:S], ks,
                                    qsb[par * D:par * D + D, 512:S],
                                    start=True, stop=True)
                            else:
                                nc.tensor.matmul(
                                    sp[:, par, lo:S], ks,
                                    qsb[par * D:par * D + D, lo:S],
                                    start=True, stop=True)
                        psb = asb.tile([P, 2, S], BF16, tag="psb", bufs=3)
                        nc.scalar.activation(out=psb[:, :, lo:S],
                                             in_=sp[:, :, lo:S],
                                             func=AF.Exp, scale=SCALE)
                        for par in range(2):
                            nc.vector.tensor_mul(
                                out=psb[:, par, lo:lo + P],
                                in0=psb[:, par, lo:lo + P], in1=masks)
                        for par in range(2):
                            h = 2 * p + par
                            a = aug[:, t, (D + 1) * h:(D + 1) * (h + 1)]
                            if lo < 512:
                                nc.tensor.matmul(
                                    ups[par][:, lo:512], a,
                                    psb[:, par, lo:512],
                                    start=(t == 0), stop=(t == 3))
                            nc.tensor.matmul(
                                ups[par][:, max(lo, 512):S], a,
                                psb[:, par, max(lo, 512):S],
                                start=(t == 0), stop=(t == NT - 1))
                    if p + 1 < NPAIR:
                        emit_qkproj(p + 1, sp)
                    # softmax denominators: fast approx reciprocal on DVE
                    # (l is in [1, ~40], no edge cases), broadcast on Pool.
                    for par in range(2):
                        linv = asb.tile([1, S], F32, tag="linv", bufs=2,
                                        name=f"linv{par}")
                        nc.vector.reciprocal_approx_fast(
                            out=linv, in_=ups[par][D:D + 1, :])
                        linvb = asb.tile([D, S], F32, tag="linvb", bufs=2)
                        nc.gpsimd.partition_broadcast(linvb, linv)
                        nc.vector.tensor_mul(
                            out=u_all[par * D:par * D + D, p, :],
                            in0=ups[par][0:D, :], in1=linvb)

            # ---------- stage 3: attn-out + residual + LN2 ----------------
            with tc.tile_pool(name="psum_ao", bufs=1, space="PSUM") as pao:
                for g in range(NT):
                    for ec in range(2):
                        ap = pao.tile([P, 512], F32, tag="apsum", bufs=2,
                                      name=f"ap{ec}")
                        for p in range(NPAIR):
                            nc.tensor.matmul(
                                ap, u_all[:, p, g * P:(g + 1) * P],
                                wvosb[:, p, ec * 512:(ec + 1) * 512],
                                start=(p == 0), stop=(p == NPAIR - 1))
                        sl = x_all[:, g, ec * 512:(ec + 1) * 512]
                        nc.vector.tensor_add(out=sl, in0=ap, in1=sl)
                    nxc = asb.tile([P, E], BF16, tag="nxc2", bufs=2)
                    _layernorm(nc, work, x_all[:, g, :], nxc, epssb)
                    for half in range(2):
                        tp = pao.tile([P, 512], BF16, tag="tp2", bufs=2)
                        for i in range(4):
                            b = 4 * half + i
                            nc.tensor.transpose(
                                tp[:, i * P:(i + 1) * P],
                                nxc[:, b * P:(b + 1) * P], ident)
                        nc.scalar.activation(
                            out=nx2T[:, 4 * half:4 * half + 4,
                                     g * P:(g + 1) * P],
                            in_=tp.rearrange("p (b m) -> p b m", b=4),
                            func=AF.Copy)

        # ---------------- stage 4: FFN -----------------------------------
        with tc.tile_pool(name="ffn_sb", bufs=1) as fsb:
            w2sb = fsb.tile([P, NF, E], BF16)
            for c in range(4):
                nc.gpsimd.dma_start(out=w2sb[:, 8 * c:8 * (c + 1), :],
                                    in_=w2d[:, 8 * c:8 * (c + 1), :])
            for sc in range(2):
                h1 = fsb.tile([P, NF, 512], BF16, tag="h1", bufs=1)
                with tc.tile_pool(name=f"psum_h{sc}", bufs=1,
                                  space="PSUM") as ph:
                    for fp in range(NF // 2):
                        w1t = fsb.tile([P, 2, NE, P], BF16, tag="w1t",
                                       bufs=3)
                        nc.sync.dma_start(
                            out=w1t,
                            in_=w1d[2 * fp:2 * fp + 2].rearrange(
                                "b p (ko m) -> p b ko m", ko=NE))
                        hp = ph.tile([P, 2, 512], F32, tag="hpsum", bufs=2)
                        for half in range(2):
                            for ek in range(NE):
                                nc.tensor.matmul(
                                    hp[:, half, :], w1t[:, half, ek, :],
                                    nx2T[:, ek, sc * 512:(sc + 1) * 512],
                                    start=(ek == 0), stop=(ek == NE - 1))
                        nc.scalar.activation(
                            out=h1[:, 2 * fp:2 * fp + 2, :], in_=hp,
                            func=AF.Relu)
                with tc.tile_pool(name=f"psum_y{sc}", bufs=1,
                                  space="PSUM") as py:
                    for st in range(4):
                        g = sc * 4 + st
                        yps = [py.tile([P, 512], F32, tag="ypsum", bufs=2, name=f"yp{ec}",
                                       ) for ec in range(2)]
                        for ft in range(NF):
                            hs = h1[:, ft, st * P:(st + 1) * P]
                            for ec in range(2):
                                nc.tensor.matmul(
                                    yps[ec],
                                    hs, w2sb[:, ft, ec * 512:(ec + 1) * 512],
                                    start=(ft == 0), stop=(ft == NF - 1))
                        for ec in range(2):
                            osb = fsb.tile([P, 512], F32, tag="osb", bufs=4)
                            nc.vector.tensor_add(
                                out=osb, in0=yps[ec],
                                in1=x_all[:, g, ec * 512:(ec + 1) * 512])
                            nc.sync.dma_start(
                                out=ov[:, g, ec * 512:(ec + 1) * 512],
                                in_=osb)


def _layernorm(nc, work, x_sl, out_ap, epssb, aug_ap=None, pool_apply=False):
    """out = (x - mean(x)) * rsqrt(var(x) + eps) as bf16.

    aug_ap: optionally also write the per-head interleaved view with a
    ones column at position D of each head (for the PV stationary) —
    done on ACT (Identity with per-partition scale/bias) to offload DVE.
    """
    stats = work.tile([P, 2, 6], F32, tag="lnstats", bufs=2)
    xg = x_sl.rearrange("p (g d) -> p g d", g=2)
    nc.vector.bn_stats(out=stats[:, 0, :], in_=xg[:, 0, :])
    nc.vector.bn_stats(out=stats[:, 1, :], in_=xg[:, 1, :])
    mv = work.tile([P, 2], F32, tag="lnmv", bufs=2)
    nc.vector.bn_aggr(out=mv, in_=stats)
    rstd = work.tile([P, 1], F32, tag="lnrstd", bufs=2)
    nc.scalar.activation(out=rstd, in_=mv[:, 1:2], func=AF.Sqrt, bias=epssb,
                         scale=1.0)
    nc.vector.reciprocal(out=rstd, in_=rstd)
    if out_ap is not None:
        nc.vector.tensor_scalar(out=out_ap, in0=x_sl, scalar1=mv[:, 0:1],
                                scalar2=rstd, op0=AL.subtract, op1=AL.mult)
    if aug_ap is not None:
        negmr = work.tile([P, 1], F32, tag="lnnegmr", bufs=2)
        nc.vector.tensor_scalar(out=negmr, in0=mv[:, 0:1], scalar1=-1.0,
                                scalar2=rstd, op0=AL.mult, op1=AL.mult)
        augv = aug_ap.rearrange("p (h e) -> p h e", h=H)
        nc.scalar.activation(
            out=augv[:, :, 0:D],
            in_=x_sl.rearrange("p (h e) -> p h e", h=H),
            func=AF.Identity, bias=negmr, scale=rstd)
        nc.vector.memset(augv[:, :, D:D + 1], 1.0)


@functools.lru_cache(maxsize=1)
def _get_program():
    return _build_program()


def _host_prep(Wq, Wk, Wv, Wo, bo, W1, b1, W2, b2, g1, beta1, g2, beta2):
    """Fold LN affines into weights; build packed bf16 per-pair weights."""
    bf = ml_dtypes.bfloat16
    g1h = g1.reshape(H, D)
    b1h = beta1.reshape(H, D)
    wqblk = np.zeros((NPAIR, P, P), np.float32)
    wkblk = np.zeros((NPAIR, P, P), np.float32)
    wvo = np.zeros((NPAIR, P, E), np.float32)
    for h in range(H):
        wqp = g1h[h][:, None] * Wq
        wkp = g1h[h][:, None] * Wk
        wvp = g1h[h][:, None] * Wv
        p, par = h // 2, h % 2
        wqblk[p, par * D:(par + 1) * D, par * D:(par + 1) * D] = wqp
        wkblk[p, par * D:(par + 1) * D, par * D:(par + 1) * D] = wkp
        wvo[p, par * D:(par + 1) * D, :] = wvp @ Wo[h * D:(h + 1) * D, :]
    # beta1 would add a constant q/k bias per head; zero for this problem.
    bq = b1h @ Wq
    bk = b1h @ Wk
    if np.abs(bq).max() > 0 or np.abs(bk).max() > 0:
        raise NotImplementedError(
            "nonzero beta1 q/k bias not supported by this kernel build")
    bvo = bo + sum((b1h[h] @ Wv) @ Wo[h * D:(h + 1) * D, :] for h in range(H))
    w1p = g2[:, None] * W1
    b1p_vec = b1 + beta2 @ W1
    if np.abs(bvo).max() > 0 or np.abs(b2).max() > 0:
        raise NotImplementedError(
            "nonzero bo/b2 residual bias not supported by this kernel build")
    if np.abs(b1p_vec).max() > 0:
        raise NotImplementedError(
            "nonzero b1/beta2 bias not supported by this kernel build")
    masks = np.triu(np.ones((P, P), np.float32))

    w1r = np.ascontiguousarray(
        w1p.reshape(NE, P, NF, P).transpose(2, 1, 0, 3).reshape(NF, P, NE * P))
    w2r = np.ascontiguousarray(W2.reshape(NF, P, E).transpose(1, 0, 2))
    return dict(
        wqblk=wqblk.astype(bf), wkblk=wkblk.astype(bf),
        wvo=wvo.astype(bf),
        w1=w1r.astype(bf), w2=w2r.astype(bf),
        masks=masks.astype(bf),
        ident=np.eye(P, dtype=bf),
    )


def kernel(x, Wq, Wk, Wv, Wo, bo, W1, b1, W2, b2, g1, beta1, g2, beta2):
    x = np.asarray(x, np.float32)
    shared = _host_prep(*(np.asarray(a, np.float32) for a in
                          (Wq, Wk, Wv, Wo, bo, W1, b1, W2, b2,
                           g1, beta1, g2, beta2)))
    nc = _get_program()
    in_maps = [dict(shared, x=np.ascontiguousarray(x[i])) for i in range(B)]
    res = run_bass_kernel_spmd(nc, in_maps, list(range(B)))
    return np.stack([res.results[i]["out"] for i in range(B)], 0)


# revision 24
# speedup vs baseline: 1.1969x; 1.1969x over previous
"""Trainium2 Bass kernel for a dense pre-norm transformer block.

Reference computation (per batch element, fp32):
    nx = LN(x; g1, beta1);  per-head q/k/v proj (shared [64,64] weights);
    causal softmax(QK^T / sqrt(1024));  out proj Wo + residual;
    nx2 = LN(x; g2, beta2);  x + relu(nx2 @ W1 + b1) @ W2 + b2.

Distribution: pure data parallel - batch B=8, one batch element per
NeuronCore, weights replicated, no collectives.

Per-core kernel strategy (matmul operands in bf16, fp32 PSUM accum):
  - LN affine (g, beta) folded into projection weights on the host.
  - Q^T/K^T per head-pair with block-diagonal [128,128] weights.
  - Scores transposed (S^T[k,q]); softmax denominator l from a ones
    column appended to the PV stationary; no max pass (scores/32 are
    O(0.1)).  exp on ACT; causal mask by 0/1 multiply on diagonal
    chunks (DVE, bf16 2x); 1/l = exp(-ln l) on ACT so the whole
    attention phase stays inside the natural_log_exp table set.
  - V never materialized: U_h = [nx_h | 1]^T @ P_h^T gives values and
    denominator in one PSUM accumulation; Wv@Wo fused on host to wvo.
  - PSUM budget (8 banks): scores+qk share one [128,2,1024] tile
    (4 banks) + two [65,1024] U accumulators (4 banks).
  - attn-out accumulated over pairs per 128-token group, immediately
    followed by residual + LN2 + transposes for that group so FFN W1
    starts while attn-out of later groups still runs.
  - FFN: h1 kept f-major in bf16; W2 resident in SBUF; W2 loop runs
    token-group-major so outputs drain early (no serial tail).
"""

import functools
import math
import os
from contextlib import ExitStack

import ml_dtypes
import numpy as np

import concourse.bass as bass
import concourse.tile as tile
from concourse import bacc, mybir
from concourse.bass_utils import run_bass_kernel_spmd

F32 = mybir.dt.float32
BF16 = mybir.dt.bfloat16
AF = mybir.ActivationFunctionType
AL = mybir.AluOpType

B, S, E, H, D, F = 8, 1024, 1024, 16, 64, 4096
P = 128
NT = S // P            # 8 token tiles
NPAIR = H // 2         # 8 head pairs
NF = F // P            # 32 f tiles
NE = E // P            # 8 e tiles
EPS = 1e-5
SCALE = 1.0 / math.sqrt(float(E))  # reference scales scores by sqrt(embed)


def _build_program():
    nc = bacc.Bacc("TRN2")

    xd = nc.dram_tensor("x", (S, E), F32, kind="ExternalInput")
    wqd = nc.dram_tensor("wqblk", (NPAIR, P, P), BF16, kind="ExternalInput")
    wkd = nc.dram_tensor("wkblk", (NPAIR, P, P), BF16, kind="ExternalInput")
    wvod = nc.dram_tensor("wvo", (NPAIR, P, E), BF16, kind="ExternalInput")
    w1d = nc.dram_tensor("w1", (NF, P, NE * P), BF16, kind="ExternalInput")
    w2d = nc.dram_tensor("w2", (P, NF, E), BF16, kind="ExternalInput")
    maskd = nc.dram_tensor("masks", (P, P), BF16, kind="ExternalInput")
    identd = nc.dram_tensor("ident", (P, P), BF16, kind="ExternalInput")
    outd = nc.dram_tensor("out", (S, E), F32, kind="ExternalOutput")

    reps = int(os.environ.get("KREP", "1"))
    with tile.TileContext(nc) as tc:
        for _ in range(reps):
            _emit(nc, tc, xd, wqd, wkd, wvod, w1d, w2d, maskd, identd, outd)
    nc.compile()
    return nc


def _emit(nc, tc, xd, wqd, wkd, wvod, w1d, w2d, maskd, identd, outd):
    xv = xd.rearrange("(t p) e -> p t e", p=P)
    ov = outd.rearrange("(t p) e -> p t e", p=P)

    with tc.tile_pool(name="consts", bufs=1) as consts, \
            tc.tile_pool(name="persist", bufs=1) as persist, \
            tc.tile_pool(name="work", bufs=1) as work:
        x_all = persist.tile([P, NT, E], F32)
        nc.sync.dma_start(out=x_all[:, 0, :], in_=xv[:, 0, :])
        ident = consts.tile([P, P], BF16)
        nc.sync.dma_start(out=ident, in_=identd[:, :])
        for t in range(1, NT):
            nc.sync.dma_start(out=x_all[:, t, :], in_=xv[:, t, :])
        masks = consts.tile([P, P], BF16)
        nc.sync.dma_start(out=masks, in_=maskd[:, :])
        wqsb = consts.tile([P, NPAIR, P], BF16)
        nc.sync.dma_start(out=wqsb, in_=wqd.rearrange("b k m -> k b m"))
        wksb = consts.tile([P, NPAIR, P], BF16)
        nc.sync.dma_start(out=wksb, in_=wkd.rearrange("b k m -> k b m"))
        epssb = consts.tile([P, 1], F32)
        nc.vector.memset(epssb, EPS)

        nx2T = persist.tile([P, NE, S], BF16)

        with tc.tile_pool(name="attn_sb", bufs=1) as asb:
            u_all = asb.tile([P, NPAIR, S], BF16)
            aug = asb.tile([P, NT, H * (D + 1)], BF16)
            nxT = asb.tile([P, NE, S], BF16)
            # attn-out weights prefetch on the Pool queue (idle early)
            wvosb = asb.tile([P, NPAIR, E], BF16)
            nc.gpsimd.dma_start(out=wvosb,
                                in_=wvod.rearrange("b k e -> k b e"))

            # ---------- stage 1: LN1 + transposes ------------------------
            # LN writes only the head-interleaved aug (ACT); per-head
            # [128,64] transposes write head parity at PSUM partition
            # offset 0/64, giving pair-stacked nxT blocks directly.
            with tc.tile_pool(name="psum_t1", bufs=1, space="PSUM") as pt1:
                for t in range(NT):
                    _layernorm(nc, work, x_all[:, t, :], None, epssb,
                               aug_ap=aug[:, t, :])
                    for half in range(2):
                        tp = pt1.tile([P, 512], BF16, tag="tp1", bufs=2)
                        for i in range(4):
                            b = 4 * half + i
                            for par in range(2):
                                h = 2 * b + par
                                nc.tensor.transpose(
                                    tp[par * D:par * D + D,
                                       i * P:(i + 1) * P],
                                    aug[:, t,
                                        (D + 1) * h:(D + 1) * h + D],
                                    ident)
                        nc.vector.tensor_copy(
                            out=nxT[:, 4 * half:4 * half + 4,
                                    t * P:(t + 1) * P],
                            in_=tp.rearrange("p (b m) -> p b m", b=4))

            # ---------- stage 2: attention per head pair ------------------
            # Queries processed in two 512-halves so score tiles are one
            # PSUM bank wide: scores double-buffer against the exp reads
            # (PE never waits for ACT), and the U accumulators double-
            # buffer across halves/pairs.  8 banks total.
            with tc.tile_pool(name="psum_at", bufs=1, space="PSUM") as pat:
                qk_sb = {}

                def emit_qkproj(pp):
                    # q/k projections; k copy on ACT (fast PSUM path),
                    # q copy on DVE, so the next pair's first score
                    # matmul unblocks quickly.
                    qsb = asb.tile([P, S], BF16, tag="qsb", bufs=2,
                                   name=f"qsb{pp % 2}")
                    ksb = asb.tile([P, S], BF16, tag="ksb", bufs=2,
                                   name=f"ksb{pp % 2}")
                    for c in range(2):
                        qk = pat.tile([P, 2, 512], F32, tag="sp", bufs=2,
                                      name="sp")
                        for i, wsb in ((0, wqsb), (1, wksb)):
                            nc.tensor.matmul(
                                qk[:, i, :], wsb[:, pp, :],
                                nxT[:, pp, c * 512:(c + 1) * 512],
                                start=True, stop=True)
                        nc.scalar.activation(
                            out=ksb[:, c * 512:(c + 1) * 512],
                            in_=qk[:, 1, :], func=AF.Copy)
                        nc.vector.tensor_copy(
                            out=qsb[:, c * 512:(c + 1) * 512],
                            in_=qk[:, 0, :])
                    qk_sb[pp] = (qsb, ksb)

                emit_qkproj(0)
                for p in range(NPAIR):
                    qsb, ksb = qk_sb.pop(p)
                    for qh in range(2):
                        q0 = qh * 512
                        ts = range(4 * qh + 4)
                        ups = [pat.tile([D + 1, 512], F32, tag=f"ups{i}",
                                        bufs=2, name=f"ups{i}")
                               for i in range(2)]
                        for t in ts:
                            lo = max(0, t * P - q0)
                            sp = pat.tile([P, 2, 512], F32, tag="sp",
                                          bufs=2, name="sp")
                            for par in range(2):
                                nc.tensor.matmul(
                                    sp[:, par, lo:512],
                                    ksb[par * D:par * D + D,
                                        t * P:(t + 1) * P],
                                    qsb[par * D:par * D + D,
                                        q0 + lo:q0 + 512],
                                    start=True, stop=True)
                            psb = asb.tile([P, 2, 512], BF16, tag="psb",
                                           bufs=4)
                            nc.scalar.activation(out=psb[:, :, lo:512],
                                                 in_=sp[:, :, lo:512],
                                                 func=AF.Exp, scale=SCALE)
                            if t * P >= q0:
                                for par in range(2):
                                    nc.vector.tensor_mul(
                                        out=psb[:, par, lo:lo + P],
                                        in0=psb[:, par, lo:lo + P],
                                        in1=masks)
                            for par in range(2):
                                h = 2 * p + par
                                nc.tensor.matmul(
                                    ups[par][:, lo:512],
                                    aug[:, t,
                                        (D + 1) * h:(D + 1) * (h + 1)],
                                    psb[:, par, lo:512],
                                    start=(t == 0), stop=(t == ts[-1]))
                        if qh == 1 and p + 1 < NPAIR:
                            emit_qkproj(p + 1)
                        # softmax denominators: fast approx reciprocal on
                        # DVE (l in [1, ~40], no edge cases), broadcast
                        # on Pool.
                        for par in range(2):
                            linv = asb.tile([1, 512], F32, tag="linv",
                                            bufs=2, name=f"linv{par}")
                            nc.vector.reciprocal_approx_fast(
                                out=linv, in_=ups[par][D:D + 1, :])
                            linvb = asb.tile([D, 512], F32, tag="linvb",
                                             bufs=2)
                            nc.gpsimd.partition_broadcast(linvb, linv)
                            nc.vector.tensor_mul(
                                out=u_all[par * D:par * D + D, p,
                                          q0:q0 + 512],
                                in0=ups[par][0:D, :], in1=linvb)

            # ---------- stage 3: attn-out + residual + LN2 ----------------
            with tc.tile_pool(name="psum_ao", bufs=1, space="PSUM") as pao:
                for g in range(NT):
                    for ec in range(2):
                        ap = pao.tile([P, 512], F32, tag="apsum", bufs=2,
                                      name=f"ap{ec}")
                        for p in range(NPAIR):
                            nc.tensor.matmul(
                                ap, u_all[:, p, g * P:(g + 1) * P],
                                wvosb[:, p, ec * 512:(ec + 1) * 512],
                                start=(p == 0), stop=(p == NPAIR - 1))
                        sl = x_all[:, g, ec * 512:(ec + 1) * 512]
                        nc.vector.tensor_add(out=sl, in0=ap, in1=sl)
                    nxc = asb.tile([P, E], BF16, tag="nxc2", bufs=2)
                    _layernorm(nc, work, x_all[:, g, :], nxc, epssb)
                    for half in range(2):
                        tp = pao.tile([P, 512], BF16, tag="tp2", bufs=2)
                        for i in range(4):
                            b = 4 * half + i
                            nc.tensor.transpose(
                                tp[:, i * P:(i + 1) * P],
                                nxc[:, b * P:(b + 1) * P], ident)
                        nc.scalar.activation(
                            out=nx2T[:, 4 * half:4 * half + 4,
                                     g * P:(g + 1) * P],
                            in_=tp.rearrange("p (b m) -> p b m", b=4),
                            func=AF.Copy)

        # ---------------- stage 4: FFN -----------------------------------
        with tc.tile_pool(name="ffn_sb", bufs=1) as fsb:
            w2sb = fsb.tile([P, NF, E], BF16)
            for c in range(4):
                nc.gpsimd.dma_start(out=w2sb[:, 8 * c:8 * (c + 1), :],
                                    in_=w2d[:, 8 * c:8 * (c + 1), :])
            for sc in range(2):
                h1 = fsb.tile([P, NF, 512], BF16, tag="h1", bufs=1)
                with tc.tile_pool(name=f"psum_h{sc}", bufs=1,
                                  space="PSUM") as ph:
                    for fp in range(NF // 2):
                        w1t = fsb.tile([P, 2, NE, P], BF16, tag="w1t",
                                       bufs=3)
                        nc.sync.dma_start(
                            out=w1t,
                            in_=w1d[2 * fp:2 * fp + 2].rearrange(
                                "b p (ko m) -> p b ko m", ko=NE))
                        hp = ph.tile([P, 2, 512], F32, tag="hpsum", bufs=2)
                        for half in range(2):
                            for ek in range(NE):
                                nc.tensor.matmul(
                                    hp[:, half, :], w1t[:, half, ek, :],
                                    nx2T[:, ek, sc * 512:(sc + 1) * 512],
                                    start=(ek == 0), stop=(ek == NE - 1))
                        nc.scalar.activation(
                            out=h1[:, 2 * fp:2 * fp + 2, :], in_=hp,
                            func=AF.Relu)
                with tc.tile_pool(name=f"psum_y{sc}", bufs=1,
                                  space="PSUM") as py:
                    for st in range(4):
                        g = sc * 4 + st
                        yps = [py.tile([P, 512], F32, tag="ypsum", bufs=2, name=f"yp{ec}",
                                       ) for ec in range(2)]
                        for ft in range(NF):
                            hs = h1[:, ft, st * P:(st + 1) * P]
                            for ec in range(2):
                                nc.tensor.matmul(
                                    yps[ec],
                                    hs, w2sb[:, ft, ec * 512:(ec + 1) * 512],
                                    start=(ft == 0), stop=(ft == NF - 1))
                        for ec in range(2):
                            osb = fsb.tile([P, 512], F32, tag="osb", bufs=4)
                            nc.vector.tensor_add(
                                out=osb, in0=yps[ec],
                                in1=x_all[:, g, ec * 512:(ec + 1) * 512])
                            nc.sync.dma_start(
                                out=ov[:, g, ec * 512:(ec + 1) * 512],
                                in_=osb)


def _layernorm(nc, work, x_sl, out_ap, epssb, aug_ap=None, pool_apply=False):
    """out = (x - mean(x)) * rsqrt(var(x) + eps) as bf16.

    aug_ap: optionally also write the per-head interleaved view with a
    ones column at position D of each head (for the PV stationary) —
    done on ACT (Identity with per-partition scale/bias) to offload DVE.
    """
    stats = work.tile([P, 2, 6], F32, tag="lnstats", bufs=2)
    xg = x_sl.rearrange("p (g d) -> p g d", g=2)
    nc.vector.bn_stats(out=stats[:, 0, :], in_=xg[:, 0, :])
    nc.vector.bn_stats(out=stats[:, 1, :], in_=xg[:, 1, :])
    mv = work.tile([P, 2], F32, tag="lnmv", bufs=2)
    nc.vector.bn_aggr(out=mv, in_=stats)
    rstd = work.tile([P, 1], F32, tag="lnrstd", bufs=2)
    nc.scalar.activation(out=rstd, in_=mv[:, 1:2], func=AF.Sqrt, bias=epssb,
                         scale=1.0)
    nc.vector.reciprocal(out=rstd, in_=rstd)
    if out_ap is not None:
        nc.vector.tensor_scalar(out=out_ap, in0=x_sl, scalar1=mv[:, 0:1],
                                scalar2=rstd, op0=AL.subtract, op1=AL.mult)
    if aug_ap is not None:
        negmr = work.tile([P, 1], F32, tag="lnnegmr", bufs=2)
        nc.vector.tensor_scalar(out=negmr, in0=mv[:, 0:1], scalar1=-1.0,
                                scalar2=rstd, op0=AL.mult, op1=AL.mult)
        augv = aug_ap.rearrange("p (h e) -> p h e", h=H)
        nc.scalar.activation(
            out=augv[:, :, 0:D],
            in_=x_sl.rearrange("p (h e) -> p h e", h=H),
            func=AF.Identity, bias=negmr, scale=rstd)
        nc.vector.memset(augv[:, :, D:D + 1], 1.0)


@functools.lru_cache(maxsize=1)
def _get_program():
    return _build_program()


def _host_prep(Wq, Wk, Wv, Wo, bo, W1, b1, W2, b2, g1, beta1, g2, beta2):
    """Fold LN affines into weights; build packed bf16 per-pair weights."""
    bf = ml_dtypes.bfloat16
    g1h = g1.reshape(H, D)
    b1h = beta1.reshape(H, D)
    wqblk = np.zeros((NPAIR, P, P), np.float32)
    wkblk = np.zeros((NPAIR, P, P), np.float32)
    wvo = np.zeros((NPAIR, P, E), np.float32)
    for h in range(H):
        wqp = g1h[h][:, None] * Wq
        wkp = g1h[h][:, None] * Wk
        wvp = g1h[h][:, None] * Wv
        p, par = h // 2, h % 2
        wqblk[p, par * D:(par + 1) * D, par * D:(par + 1) * D] = wqp
        wkblk[p, par * D:(par + 1) * D, par * D:(par + 1) * D] = wkp
        wvo[p, par * D:(par + 1) * D, :] = wvp @ Wo[h * D:(h + 1) * D, :]
    # beta1 would add a constant q/k bias per head; zero for this problem.
    bq = b1h @ Wq
    bk = b1h @ Wk
    if np.abs(bq).max() > 0 or np.abs(bk).max() > 0:
        raise NotImplementedError(
            "nonzero beta1 q/k bias not supported by this kernel build")
    bvo = bo + sum((b1h[h] @ Wv) @ Wo[h * D:(h + 1) * D, :] for h in range(H))
    w1p = g2[:, None] * W1
    b1p_vec = b1 + beta2 @ W1
    if np.abs(bvo).max() > 0 or np.abs(b2).max() > 0:
        raise NotImplementedError(
            "nonzero bo/b2 residual bias not supported by this kernel build")
    if np.abs(b1p_vec).max() > 0:
        raise NotImplementedError(
            "nonzero b1/beta2 bias not supported by this kernel build")
    masks = np.triu(np.ones((P, P), np.float32))

    w1r = np.ascontiguousarray(
        w1p.reshape(NE, P, NF, P).transpose(2, 1, 0, 3).reshape(NF, P, NE * P))
    w2r = np.ascontiguousarray(W2.reshape(NF, P, E).transpose(1, 0, 2))
    return dict(
        wqblk=wqblk.astype(bf), wkblk=wkblk.astype(bf),
        wvo=wvo.astype(bf),
        w1=w1r.astype(bf), w2=w2r.astype(bf),
        masks=masks.astype(bf),
        ident=np.eye(P, dtype=bf),
    )


def kernel(x, Wq, Wk, Wv, Wo, bo, W1, b1, W2, b2, g1, beta1, g2, beta2):
    x = np.asarray(x, np.float32)
    shared = _host_prep(*(np.asarray(a, np.float32) for a in
                          (Wq, Wk, Wv, Wo, bo, W1, b1, W2, b2,
                           g1, beta1, g2, beta2)))
    nc = _get_program()
    in_maps = [dict(shared, x=np.ascontiguousarray(x[i])) for i in range(B)]
    res = run_bass_kernel_spmd(nc, in_maps, list(range(B)))
    return np.stack([res.results[i]["out"] for i in range(B)], 0)
